# revision 27
# baseline (speedup 1.0000x reference)
"""Trainium2 Bass kernel for a pre-LN transformer block (B=4, T=2048, D=1024,
H=16, HS=64, FF=4096, causal attention).

Sharding: data-parallel over batches x 2-way tensor-parallel attention
(8 heads/core over all T) -> pair ReduceScatter of the attention-output
projection over the sequence dim -> sequence-parallel FFN (full FF width,
T/2 rows per core).  No AllReduce anywhere; each core emits the final
output for its own T/2 rows.

Core c (0..7): batch b = c//2, half = c%2.  half h owns t-slices
[ci*512 + h*256, ci*512 + h*256 + 256) for ci in 0..3.

Layout: activations feature-major (d on partitions, t on free dim).
LayerNorm gains are folded into the weights on the host; LN on-chip is
just (x - mu) * inv_sigma with stats from DVE adder trees + one-column
matmuls that share the projection PSUM slots.  Attention is
phase-separated per (head-pair, chunk): score matmuls run a few steps
ahead of the o-accum matmuls with exp ([128,2,512] double-bank ACT ops)
in between, so the PE never stalls behind the scalar engine.
"""

import numpy as np
import ml_dtypes

import concourse.bacc as bacc
import concourse.bass as bass
import concourse.mybir as mybir
import concourse.tile as tile
from concourse.bass_utils import run_bass_kernel_spmd

BF16NP = ml_dtypes.bfloat16

B, T, D, H, HS, FF = 4, 2048, 1024, 16, 64, 4096
EPS = 1e-5
NCORES = 8
TP = 2
LH = H // TP          # 8 local heads
LHE = LH * HS         # 512 local head-embed width
LT = T // TP          # 1024 local rows (FFN/output)
KD = D // 128         # 8 d k-tiles
KHE = LHE // 128      # 4 he k-tiles
KFF = FF // 128       # 32 ff tiles
NCH = T // 512        # 4 t-chunks of 512
NST = T // 128        # 16 s-tiles of 128
PAIRS = [[0, 1], [2, 3], [4, 5], [6, 7]]
OA_LAG = 2            # psc tiles in flight between scores and o-accum

F32 = mybir.dt.float32
BF = mybir.dt.bfloat16


def _ln_stats(nc, pool, psum_pool, psum_tag, src, ones_col, psum_bufs=2,
              sq_on_act=False, bcast_mm=None):
    """src: [128, KD, 512] AP.  Returns (Ab, Bb) [128,512] bf16 broadcast
    tiles so that xn = src*Ab + Bb.  Stats via bf16 DVE adder trees plus
    two one-column matmuls that borrow slots from psum_pool/psum_tag.
    bcast_mm=(ones_row, pool, tag, bufs) broadcasts via PE matmul into
    PSUM instead of gpsimd partition_broadcast."""
    mm = nc.tensor.matmul
    Alu = mybir.AluOpType
    Act = mybir.ActivationFunctionType

    def lvl(tg, n):
        return pool.tile([128, 512], BF, tag=f"{tg}{n}", bufs=2, name=tg)

    s2 = []
    for i in range(4):
        s = lvl("lts", 2)
        nc.vector.tensor_add(out=s, in0=src[:, 2 * i, :],
                             in1=src[:, 2 * i + 1, :])
        s2.append(s)
    s4 = []
    for i in range(2):
        s = lvl("lts", 4)
        nc.vector.tensor_add(out=s, in0=s2[2 * i], in1=s2[2 * i + 1])
        s4.append(s)
    s_all = lvl("lts", 8)
    nc.vector.tensor_add(out=s_all, in0=s4[0], in1=s4[1])
    ps_s = psum_pool.tile([1, 512], F32, tag=psum_tag, bufs=psum_bufs,
                          name="ps_s")
    mm(out=ps_s, lhsT=ones_col, rhs=s_all, start=True, stop=True)

    q2 = []
    for i in range(4):
        sqa = pool.tile([128, 512], BF, tag="ln_sq", bufs=2, name="ln_sq")
        sqb = pool.tile([128, 512], BF, tag="ln_sq", bufs=2, name="ln_sq")
        if sq_on_act:
            nc.scalar.square(out=sqa, in_=src[:, 2 * i, :])
            nc.scalar.square(out=sqb, in_=src[:, 2 * i + 1, :])
        else:
            nc.vector.tensor_mul(out=sqa, in0=src[:, 2 * i, :],
                                 in1=src[:, 2 * i, :])
            nc.vector.tensor_mul(out=sqb, in0=src[:, 2 * i + 1, :],
                                 in1=src[:, 2 * i + 1, :])
        q = lvl("ltq", 2)
        nc.vector.tensor_add(out=q, in0=sqa, in1=sqb)
        q2.append(q)
    q4_ = []
    for i in range(2):
        q = lvl("ltq", 4)
        nc.vector.tensor_add(out=q, in0=q2[2 * i], in1=q2[2 * i + 1])
        q4_.append(q)
    q_all = lvl("ltq", 8)
    nc.vector.tensor_add(out=q_all, in0=q4_[0], in1=q4_[1])
    ps_q = psum_pool.tile([1, 512], F32, tag=psum_tag, bufs=psum_bufs,
                          name="ps_q")
    mm(out=ps_q, lhsT=ones_col, rhs=q_all, start=True, stop=True)

    m = pool.tile([1, 512], F32, tag="ln_m", bufs=1, name="ln_m")
    e2 = pool.tile([1, 512], F32, tag="ln_e2", bufs=1, name="ln_e2")
    nc.vector.tensor_scalar_mul(out=m, in0=ps_s, scalar1=1.0 / D)
    nc.vector.tensor_scalar_mul(out=e2, in0=ps_q, scalar1=1.0 / D)
    msq = pool.tile([1, 512], F32, tag="ln_msq", bufs=1, name="ln_msq")
    nc.vector.tensor_mul(out=msq, in0=m, in1=m)
    var = pool.tile([1, 512], F32, tag="ln_var", bufs=1, name="ln_var")
    nc.vector.scalar_tensor_tensor(out=var, in0=e2, scalar=EPS, in1=msq,
                                   op0=Alu.add, op1=Alu.subtract)
    sd = pool.tile([1, 512], F32, tag="ln_sd", bufs=1, name="ln_sd")
    nc.scalar.activation(out=sd, in_=var, func=Act.Sqrt)
    a_row = pool.tile([1, 512], F32, tag="ln_a", bufs=1, name="ln_a")
    nc.vector.reciprocal_approx_fast(out=a_row, in_=sd)
    b_row = pool.tile([1, 512], F32, tag="ln_b", bufs=1, name="ln_b")
    nc.vector.scalar_tensor_tensor(out=b_row, in0=m, scalar=-1.0, in1=a_row,
                                   op0=Alu.mult, op1=Alu.mult)
    ac = pool.tile([1, 512], BF, tag="ln_ac", bufs=1, name="ln_ac")
    bc = pool.tile([1, 512], BF, tag="ln_bc", bufs=1, name="ln_bc")
    nc.vector.tensor_copy(out=ac, in_=a_row)
    nc.vector.tensor_copy(out=bc, in_=b_row)
    if bcast_mm is not None:
        # broadcast via K=1 matmul on the PE (avoids the gpsimd queue)
        ones_row, bpool, btag, bbufs = bcast_mm
        Ab = bpool.tile([128, 512], F32, tag=btag, bufs=bbufs, name="ln_Abp")
        Bb = bpool.tile([128, 512], F32, tag=btag, bufs=bbufs, name="ln_Bbp")
        mm(out=Ab, lhsT=ones_row, rhs=ac, start=True, stop=True)
        mm(out=Bb, lhsT=ones_row, rhs=bc, start=True, stop=True)
        return Ab, Bb
    Ab = pool.tile([128, 512], BF, tag="ln_Ab", bufs=2, name="ln_Ab")
    Bb = pool.tile([128, 512], BF, tag="ln_Bb", bufs=2, name="ln_Bb")
    nc.gpsimd.partition_broadcast(Ab, ac)
    nc.gpsimd.partition_broadcast(Bb, bc)
    return Ab, Bb


def _ln_apply(nc, pool, src_k, Ab, Bb, out_slice):
    """out = src*Ab + Bb (bf16)."""
    t1 = pool.tile([128, 512], BF, tag="ln_t1", bufs=2, name="ln_t1")
    nc.vector.tensor_mul(out=t1, in0=src_k, in1=Ab)
    nc.vector.tensor_add(out=out_slice, in0=t1, in1=Bb)


def _emit(nc, tc, t):
    mm = nc.tensor.matmul
    Alu = mybir.AluOpType
    Act = mybir.ActivationFunctionType

    xT_v = t["xT"].rearrange("(k p) t -> p k t", p=128)
    xres_v = t["xresT"].rearrange("(k p) t -> p k t", p=128)
    w1_v = t["w1"].rearrange("(k p) e -> p k e", p=128)
    w2_v = t["w2"].rearrange("(k p) e -> p k e", p=128)
    outT_v = t["outT"]

    # ---------------- persistent pools ----------------
    dram = tc.alloc_tile_pool(name="dram", bufs=1, space="DRAM")
    rs_in = [dram.tile([TP, D, 256], BF, name=f"rsi{c}") for c in range(NCH)]
    rs_out = [dram.tile([D, 256], BF, name=f"rso{c}") for c in range(NCH)]

    consts = tc.alloc_tile_pool(name="consts", bufs=1)
    ones_col = consts.tile([128, 1], BF)
    nc.vector.memset(ones_col, 1.0)
    ones_row = consts.tile([1, 128], BF)
    nc.vector.memset(ones_row, 1.0)

    bq_sb = consts.tile([128, KHE], F32)
    bk_sb = consts.tile([128, KHE], F32)
    bo2_sb = consts.tile([128, KD], F32)
    b2_sb = consts.tile([128, KD], F32)
    b1_sb = consts.tile([128, KFF], F32)
    for name, dst in (("bq", bq_sb), ("bk", bk_sb),
                      ("bo2", bo2_sb), ("b2", b2_sb)):
        nc.sync.dma_start(out=dst, in_=t[name].rearrange("(k p) -> p k", p=128))
    nc.sync.dma_start(out=b1_sb, in_=t["b1f"].rearrange("(k p) -> p k", p=128))
    # v bias broadcast over all partitions: [128, LHE]
    bvb = consts.tile([128, LHE], BF)
    bv_row = consts.tile([1, LHE], BF)
    nc.gpsimd.dma_start(out=bv_row,
                        in_=t["bv"].rearrange("(o e) -> o e", o=1))
    nc.gpsimd.partition_broadcast(bvb, bv_row)

    wlate = tc.alloc_tile_pool(name="wlate", bufs=1)
    wo_sb = wlate.tile([128, KHE, D], BF, tag="wo")
    nc.scalar.dma_start(out=wo_sb,
                        in_=t["wo"].rearrange("(k p) e -> p k e", p=128))

    # ================= Stage A: LN1 + QKV + attention + Wo + RS ============
    with tc.tile_pool(name="abc", bufs=1) as ab:
        kT = ab.tile([128, LH // 2, T], BF, tag="kT")
        qT = ab.tile([128, LH // 2, T], BF, tag="qT")
        vS = ab.tile([128, NST, LH * 128], BF, tag="vS")
        nc.vector.memset(vS, 0.0)
        for h in range(LH):
            nc.vector.memset(vS[:, :, h * 128 + 64:h * 128 + 65], 1.0)
        wq_sb = ab.tile([128, KD, LHE], BF, tag="wq")
        wk_sb = ab.tile([128, KD, LHE], BF, tag="wk")
        wv_sb = ab.tile([128, KD, LHE], BF, tag="wv")

        def load_qkv_weights():
            for eng, srct, dst in ((nc.scalar, t["wq"], wq_sb),
                                   (nc.scalar, t["wk"], wk_sb),
                                   (nc.sync, t["wv"], wv_sb)):
                eng.dma_start(out=dst,
                              in_=srct.rearrange("(k p) e -> p k e", p=128))

        with tc.tile_pool(name="projpsum", bufs=2, space="PSUM") as pproj, \
             tc.tile_pool(name="scpsum", bufs=OA_LAG, space="PSUM") as psc, \
             tc.tile_pool(name="oapsum", bufs=2, space="PSUM") as poa:

            def ln1(ci):
                xf = ab.tile([128, KD, 512], F32, tag="xf", bufs=2, name="xf")
                for k in range(KD):
                    eng = nc.sync if k % 2 == 0 else nc.scalar
                    eng.dma_start(out=xf[:, k:k + 1, :],
                                  in_=xT_v[:, k:k + 1,
                                           ci * 512:ci * 512 + 512])
                Ab, Bb = _ln_stats(nc, ab, pproj, "ps_proj", xf, ones_col)
                hT = ab.tile([128, KD, 512], BF, tag="hT", bufs=2, name="hT")
                for k in range(KD):
                    _ln_apply(nc, ab, xf[:, k, :], Ab, Bb, hT[:, k, :])
                return hT

            def proj_units(ci, hT):
                c0 = ci * 512
                units = []
                for w_sb, dst, bias in ((wk_sb, kT, bk_sb), (wq_sb, qT, bq_sb)):
                    for et in range(LH // 2):
                        def u(w_sb=w_sb, dst=dst, bias=bias, et=et, hT=hT,
                              c0=c0):
                            ps = pproj.tile([128, 512], F32, tag="ps_proj",
                                            bufs=2, name="ps_proj")
                            for k in range(KD):
                                mm(out=ps,
                                   lhsT=w_sb[:, k, et * 128:(et + 1) * 128],
                                   rhs=hT[:, k, :],
                                   start=(k == 0), stop=(k == KD - 1))
                            nc.vector.tensor_scalar(
                                out=dst[:, et, c0:c0 + 512], in0=ps,
                                scalar1=bias[:, et:et + 1], scalar2=None,
                                op0=Alu.add)
                        units.append(u)
                for sti in range(4):
                    st = ci * 4 + sti
                    def u(sti=sti, st=st, hT=hT):
                        ps = pproj.tile([128, LHE], F32, tag="ps_proj", bufs=2,
                                        name="ps_v")
                        for k in range(KD):
                            mm(out=ps,
                               lhsT=hT[:, k, sti * 128:sti * 128 + 128],
                               rhs=wv_sb[:, k, :],
                               start=(k == 0), stop=(k == KD - 1))
                        nc.vector.tensor_add(
                            out=vS[:, st, :].rearrange("p (h e) -> p h e",
                                                       h=LH)[:, :, 0:64],
                            in0=ps.rearrange("p (h e) -> p h e", e=64),
                            in1=bvb.rearrange("p (h e) -> p h e", e=64))
                    units.append(u)
                return units

            def attention(ci, oT, pending):
                c0 = ci * 512
                nb = 4 * (ci + 1)
                total_steps = (LH // 2) * (nb + OA_LAG)
                spacing = (max(1, total_steps // len(pending))
                           if pending else 0)
                stepctr = 0
                for hp in range(LH // 2):
                    po = [poa.tile([128, 512], F32, tag="po", bufs=2,
                                   name="po")
                          for _ in range(2)]
                    exs = [None] * nb

                    def scores(sb):
                        s0 = sb * 128
                        ps2 = psc.tile([128, 2, 512], F32, tag="ps_sc",
                                       bufs=OA_LAG, name="ps_sc")
                        for hi in range(2):
                            mm(out=ps2[:, hi, :],
                               lhsT=kT[hi * 64:hi * 64 + 64, hp, s0:s0 + 128],
                               rhs=qT[hi * 64:hi * 64 + 64, hp, c0:c0 + 512],
                               start=True, stop=True)
                        ex = ab.tile([128, 2, 512], BF, tag="ex",
                                     bufs=OA_LAG + 1, name="ex")
                        nc.scalar.activation(out=ex, in_=ps2, func=Act.Exp)
                        midx = sb - 4 * ci
                        if midx >= 0:
                            for hi in range(2):
                                nc.gpsimd.affine_select(
                                    out=ex[:, hi, :], in_=ex[:, hi, :],
                                    compare_op=Alu.is_ge, fill=0.0,
                                    base=-(midx * 128), channel_multiplier=-1,
                                    pattern=[[1, 512]])
                        exs[sb] = ex

                    def oacc(sb):
                        for hi in range(2):
                            h_loc = hp * 2 + hi
                            mm(out=po[hi],
                               lhsT=vS[:, sb,
                                       h_loc * 128:h_loc * 128 + 128],
                               rhs=exs[sb][:, hi, :],
                               start=(sb == 0), stop=(sb == nb - 1))

                    for step in range(nb + OA_LAG):
                        if step < nb:
                            scores(step)
                        if step >= OA_LAG:
                            oacc(step - OA_LAG)
                        stepctr += 1
                        if pending and stepctr % spacing == 0:
                            pending.pop(0)()

                    for hi in range(2):
                        h_loc = hp * 2 + hi
                        dnr = ab.tile([1, 512], F32, tag="dnr", bufs=1,
                                      name="dnr")
                        nc.vector.tensor_copy(out=dnr, in_=po[hi][64:65, :])
                        rcp = ab.tile([1, 512], F32, tag="rcp", bufs=2,
                                      name="rcp")
                        nc.vector.reciprocal_approx_fast(out=rcp, in_=dnr)
                        bc = ab.tile([64, 512], F32, tag="bc", bufs=1,
                                     name="bc")
                        nc.gpsimd.partition_broadcast(bc, rcp)
                        nc.vector.tensor_mul(
                            out=oT[hi * 64:hi * 64 + 64, hp, :],
                            in0=po[hi][0:64, :], in1=bc)
                while pending:
                    pending.pop(0)()

            def wo_units(ci, oT):
                rsv = rs_in[ci].rearrange("j (k p) t -> j k p t", p=128)
                units = []
                for dt in range(KD):
                    def u(dt=dt, oT=oT, rsv=rsv, last=(ci == NCH - 1)):
                        ps = pproj.tile([128, 512], F32, tag="ps_proj",
                                        bufs=2, name="ps_wo")
                        for k in range(KHE):
                            mm(out=ps,
                               lhsT=wo_sb[:, k, dt * 128:(dt + 1) * 128],
                               rhs=oT[:, k, :],
                               start=(k == 0), stop=(k == KHE - 1))
                        stg = ab.tile([128, 512], BF, tag="stg1", bufs=2,
                                      name="stg1")
                        if last:
                            nc.scalar.activation(
                                out=stg, in_=ps, func=Act.Identity,
                                bias=bo2_sb[:, dt:dt + 1])
                        else:
                            nc.vector.tensor_scalar(
                                out=stg, in0=ps, scalar1=bo2_sb[:, dt:dt + 1],
                                scalar2=None, op0=Alu.add)
                        for j in range(TP):
                            nc.sync.dma_start(
                                out=rsv[j, dt, :, :],
                                in_=stg[:, j * 256:(j + 1) * 256])
                    units.append(u)
                return units

            def rs_issue(ci):
                nc.gpsimd.collective_compute(
                    "ReduceScatter", Alu.add, replica_groups=PAIRS,
                    ins=[rs_in[ci].opt()], outs=[rs_out[ci].opt()])

            hTs = {0: ln1(0)}
            load_qkv_weights()
            for u in proj_units(0, hTs[0]):
                u()
            oTs = {}
            for ci in range(NCH):
                pending = []
                if ci >= 1:
                    pending += wo_units(ci - 1, oTs[ci - 1])
                if ci + 1 < NCH:
                    hTs[ci + 1] = ln1(ci + 1)
                    pending += proj_units(ci + 1, hTs[ci + 1])
                oTs[ci] = ab.tile([128, KHE, 512], BF, tag="oT", bufs=2,
                                  name="oT")
                attention(ci, oTs[ci], pending)
                if ci >= 1:
                    rs_issue(ci - 1)
            for u in wo_units(NCH - 1, oTs[NCH - 1]):
                u()
            rs_issue(NCH - 1)

    # ================= Stage B: residual + LN2 + FFN (own T/2 rows) ========
    with tc.tile_pool(name="de", bufs=1) as de, \
         tc.tile_pool(name="upsum", bufs=3, space="PSUM") as pu, \
         tc.tile_pool(name="fpsum", bufs=2, space="PSUM") as pf:

        w2_sb = de.tile([128, KFF, D], BF, tag="w2t", bufs=1, name="w2t")

        def prep(lc):
            c0 = lc * 512
            # residual: xmid = xres + rs_out  (bf16 residual stream)
            xr = de.tile([128, KD, 512], BF, tag="xr", bufs=1, name="xr")
            nc.sync.dma_start(out=xr[:, 0:4, :],
                              in_=xres_v[:, 0:4, c0:c0 + 512])
            nc.scalar.dma_start(out=xr[:, 4:8, :],
                                in_=xres_v[:, 4:8, c0:c0 + 512])
            arr = de.tile([128, KD, 2, 256], BF, tag="arr", bufs=1, name="arr")
            rsv0 = rs_out[2 * lc].rearrange("(k p) t -> p k t", p=128)
            rsv1 = rs_out[2 * lc + 1].rearrange("(k p) t -> p k t", p=128)
            nc.gpsimd.dma_start(out=arr[:, :, 0, :], in_=rsv0)
            nc.sync.dma_start(out=arr[:, 0:4, 1, :], in_=rsv1[:, 0:4, :])
            nc.scalar.dma_start(out=arr[:, 4:8, 1, :], in_=rsv1[:, 4:8, :])
            xmid = de.tile([128, KD, 512], BF, tag="xmid", bufs=2,
                           name="xmid")
            for k in range(KD):
                nc.vector.tensor_add(
                    out=xmid[:, k, :], in0=xr[:, k, :],
                    in1=arr[:, k, :, :].rearrange("p j t -> p (j t)"))
            # LN2 (gains folded into W1/b1f on host)
            Ab2, Bb2 = _ln_stats(nc, de, pu, "ps_u", xmid, ones_col,
                                 psum_bufs=3,
                                 bcast_mm=(ones_row, pf, "ps_f", 3))
            h2 = de.tile([128, KD, 512], BF, tag="h2", bufs=1, name="h2")
            for k in range(KD):
                _ln_apply(nc, de, xmid[:, k, :], Ab2, Bb2, h2[:, k, :])
            return xmid, h2

        def fc1(h2):
            # FFN up: u = relu(h2 @ W1 + b1f)   (relu+bias on DVE)
            u = de.tile([128, KFF, 512], BF, tag="u", bufs=1, name="u")
            for q16 in range(16):
                w1t = de.tile([128, KD, 256], BF, tag="w1t", bufs=2,
                              name="w1t")
                nc.sync.dma_start(out=w1t,
                                  in_=w1_v[:, :, q16 * 256:(q16 + 1) * 256])
                for fi in range(2):
                    fft = q16 * 2 + fi
                    ps = pu.tile([128, 512], F32, tag="ps_u", bufs=3,
                                 name="ps_u")
                    for k in range(KD):
                        mm(out=ps,
                           lhsT=w1t[:, k, fi * 128:fi * 128 + 128],
                           rhs=h2[:, k, :],
                           start=(k == 0), stop=(k == KD - 1))
                    nc.vector.tensor_scalar(
                        out=u[:, fft, :], in0=ps,
                        scalar1=b1_sb[:, fft:fft + 1], scalar2=0.0,
                        op0=Alu.add, op1=Alu.max)
            return u

        def fc2(lc, u, xmid):
            c0 = lc * 512
            # FFN down + bias + residual -> store
            for dt in range(KD):
                ps = pf.tile([128, 512], F32, tag="ps_f", bufs=3, name="ps_f")
                for k2 in range(KFF):
                    mm(out=ps,
                       lhsT=w2_sb[:, k2, dt * 128:(dt + 1) * 128],
                       rhs=u[:, k2, :],
                       start=(k2 == 0), stop=(k2 == KFF - 1))
                o_f = de.tile([128, 512], F32, tag="o_f", bufs=2, name="o_f")
                nc.vector.scalar_tensor_tensor(
                    out=o_f, in0=ps, scalar=b2_sb[:, dt:dt + 1],
                    in1=xmid[:, dt, :], op0=Alu.add, op1=Alu.add)
                nc.sync.dma_start(
                    out=outT_v[dt * 128:(dt + 1) * 128, c0:c0 + 512],
                    in_=o_f)

        xmid0, h20 = prep(0)
        u0 = fc1(h20)
        for q4 in range(4):
            nc.gpsimd.dma_start(
                out=w2_sb[:, q4 * 8:(q4 + 1) * 8, :],
                in_=w2_v[:, q4 * 8:(q4 + 1) * 8, :])
        xmid1, h21 = prep(1)
        fc2(0, u0, xmid0)
        u1 = fc1(h21)
        fc2(1, u1, xmid1)

    wlate.release()
    consts.release()
    dram.release()


def _build():
    nc = bacc.Bacc("TRN2", target_bir_lowering=False, debug=False,
                   num_devices=NCORES)

    tensors = {}
    tensors["xT"] = nc.dram_tensor("xT", [D, T], F32, kind="ExternalInput").ap()
    tensors["xresT"] = nc.dram_tensor("xresT", [D, LT], BF,
                                      kind="ExternalInput").ap()
    for name, shape, dt in (
        ("wq", [D, LHE], BF), ("wk", [D, LHE], BF), ("wv", [D, LHE], BF),
        ("wo", [LHE, D], BF), ("w1", [D, FF], BF), ("w2", [FF, D], BF),
        ("bq", [LHE], F32), ("bk", [LHE], F32), ("bv", [LHE], F32),
        ("b1f", [FF], F32), ("bo2", [D], F32), ("b2", [D], F32),
    ):
        tensors[name] = nc.dram_tensor(name, shape, dt,
                                       kind="ExternalInput").ap()
    tensors["outT"] = nc.dram_tensor("out", [D, LT], F32,
                                     kind="ExternalOutput").ap()

    with tile.TileContext(nc, num_cores=NCORES) as tc:
        _emit(nc, tc, tensors)

    nc.compile()
    return nc


_NC_CACHE = None


def _get_nc():
    global _NC_CACHE
    if _NC_CACHE is None:
        _NC_CACHE = _build()
    return _NC_CACHE


def _shard_inputs(x, Wq, Wk, Wv, Wo, bo, W1, b1, W2, b2, g1, be1, g2, be2):
    """Build the 8 per-core input maps (LN gains folded into weights)."""
    bf = lambda a: np.ascontiguousarray(a).astype(BF16NP)
    f32 = lambda a: np.ascontiguousarray(a, dtype=np.float32)

    x = np.asarray(x, dtype=np.float32)
    Wq = np.asarray(Wq, dtype=np.float32)
    Wk = np.asarray(Wk, dtype=np.float32)
    Wv = np.asarray(Wv, dtype=np.float32)
    Wo = np.asarray(Wo, dtype=np.float32)
    W1 = np.asarray(W1, dtype=np.float32)
    W2 = np.asarray(W2, dtype=np.float32)
    g1 = np.asarray(g1, dtype=np.float32)
    be1 = np.asarray(be1, dtype=np.float32)
    g2 = np.asarray(g2, dtype=np.float32)
    be2 = np.asarray(be2, dtype=np.float32)
    b1 = np.asarray(b1, dtype=np.float32)

    scale = float(HS) ** -0.5
    # fold g1 into QKV weights, be1 into QKV biases; fold the score scale
    # into Wq/bq.  Per-head [H, D, HS] -> concat heads -> [D, H*HS].
    wq_f = (g1[None, :, None] * Wq).transpose(1, 0, 2).reshape(D, D) * scale
    wk_f = (g1[None, :, None] * Wk).transpose(1, 0, 2).reshape(D, D)
    wv_f = (g1[None, :, None] * Wv).transpose(1, 0, 2).reshape(D, D)
    bq_f = np.einsum("d,hde->he", be1, Wq).reshape(D) * scale
    bk_f = np.einsum("d,hde->he", be1, Wk).reshape(D)
    bv_f = np.einsum("d,hde->he", be1, Wv).reshape(D)
    # fold g2/be2 into W1/b1
    w1_f = g2[:, None] * W1
    b1_f = b1 + be2 @ W1

    in_maps = []
    for c in range(NCORES):
        b, half = divmod(c, TP)
        hes = slice(half * LHE, (half + 1) * LHE)
        xt = x[b].T
        xres = np.concatenate(
            [xt[:, ci * 512 + half * 256: ci * 512 + half * 256 + 256]
             for ci in range(NCH)], axis=1)
        in_maps.append({
            "xT": f32(xt),
            "xresT": bf(xres),
            "wq": bf(wq_f[:, hes]), "wk": bf(wk_f[:, hes]),
            "wv": bf(wv_f[:, hes]),
            "bq": f32(bq_f[hes]), "bk": f32(bk_f[hes]), "bv": f32(bv_f[hes]),
            "wo": bf(Wo[hes, :]),
            "bo2": f32(np.asarray(bo, dtype=np.float32) / TP),
            "w1": bf(w1_f), "b1f": f32(b1_f),
            "w2": bf(W2), "b2": f32(np.asarray(b2, dtype=np.float32)),
        })
    return in_maps


def kernel(x, Wq, Wk, Wv, Wo, bo, W1, b1, W2, b2, g1, be1, g2, be2,
           _trace=False):
    nc = _get_nc()
    in_maps = _shard_inputs(x, Wq, Wk, Wv, Wo, bo, W1, b1, W2, b2,
                            g1, be1, g2, be2)
    res = run_bass_kernel_spmd(nc, in_maps, list(range(NCORES)),
                               trace=_trace)
    out = np.empty((B, T, D), dtype=np.float32)
    for b in range(B):
        for half in range(TP):
            o = res.results[TP * b + half]["out"]  # [D, LT]
            for ci in range(NCH):
                t0 = ci * 512 + half * 256
                out[b, t0:t0 + 256, :] = o[:, ci * 256:(ci + 1) * 256].T
    if _trace:
        kernel.last_exec_time_ns = res.exec_time_ns
        kernel.last_results = res
    return out


# revision 28
# speedup vs baseline: 1.0015x; 1.0015x over previous
"""Trainium2 Bass kernel for a pre-LN transformer block (B=4, T=2048, D=1024,
H=16, HS=64, FF=4096, causal attention).

Sharding: data-parallel over batches x 2-way tensor-parallel attention
(8 heads/core over all T) -> pair ReduceScatter of the attention-output
projection over the sequence dim -> sequence-parallel FFN (full FF width,
T/2 rows per core).  No AllReduce anywhere; each core emits the final
output for its own T/2 rows.

Core c (0..7): batch b = c//2, half = c%2.  half h owns t-slices
[ci*512 + h*256, ci*512 + h*256 + 256) for ci in 0..3.

Layout: activations feature-major (d on partitions, t on free dim).
LayerNorm gains are folded into the weights on the host; LN on-chip is
just (x - mu) * inv_sigma with stats from DVE adder trees + one-column
matmuls that share the projection PSUM slots.  Attention is
phase-separated per (head-pair, chunk): score matmuls run a few steps
ahead of the o-accum matmuls with exp ([128,2,512] double-bank ACT ops)
in between, so the PE never stalls behind the scalar engine.
"""

import numpy as np
import ml_dtypes

import concourse.bacc as bacc
import concourse.bass as bass
import concourse.mybir as mybir
import concourse.tile as tile
from concourse.bass_utils import run_bass_kernel_spmd

BF16NP = ml_dtypes.bfloat16

B, T, D, H, HS, FF = 4, 2048, 1024, 16, 64, 4096
EPS = 1e-5
NCORES = 8
TP = 2
LH = H // TP          # 8 local heads
LHE = LH * HS         # 512 local head-embed width
LT = T // TP          # 1024 local rows (FFN/output)
KD = D // 128         # 8 d k-tiles
KHE = LHE // 128      # 4 he k-tiles
KFF = FF // 128       # 32 ff tiles
NCH = T // 512        # 4 t-chunks of 512
NST = T // 128        # 16 s-tiles of 128
PAIRS = [[0, 1], [2, 3], [4, 5], [6, 7]]
OA_LAG = 2            # psc tiles in flight between scores and o-accum

F32 = mybir.dt.float32
BF = mybir.dt.bfloat16


def _ln_stats(nc, pool, psum_pool, psum_tag, src, ones_col, psum_bufs=2,
              sq_on_act=False, bcast_mm=None):
    """src: [128, KD, 512] AP.  Returns (Ab, Bb) [128,512] bf16 broadcast
    tiles so that xn = src*Ab + Bb.  Stats via bf16 DVE adder trees plus
    two one-column matmuls that borrow slots from psum_pool/psum_tag.
    bcast_mm=(ones_row, pool, tag, bufs) broadcasts via PE matmul into
    PSUM instead of gpsimd partition_broadcast."""
    mm = nc.tensor.matmul
    Alu = mybir.AluOpType
    Act = mybir.ActivationFunctionType

    def lvl(tg, n):
        return pool.tile([128, 512], BF, tag=f"{tg}{n}", bufs=2, name=tg)

    s2, q2 = [], []
    for i in range(4):
        s = lvl("lts", 2)
        nc.vector.tensor_add(out=s, in0=src[:, 2 * i, :],
                             in1=src[:, 2 * i + 1, :])
        s2.append(s)
        sqa = pool.tile([128, 512], BF, tag="ln_sq", bufs=2, name="ln_sq")
        sqb = pool.tile([128, 512], BF, tag="ln_sq", bufs=2, name="ln_sq")
        if sq_on_act:
            nc.scalar.square(out=sqa, in_=src[:, 2 * i, :])
            nc.scalar.square(out=sqb, in_=src[:, 2 * i + 1, :])
        else:
            nc.vector.tensor_mul(out=sqa, in0=src[:, 2 * i, :],
                                 in1=src[:, 2 * i, :])
            nc.vector.tensor_mul(out=sqb, in0=src[:, 2 * i + 1, :],
                                 in1=src[:, 2 * i + 1, :])
        q = lvl("ltq", 2)
        nc.vector.tensor_add(out=q, in0=sqa, in1=sqb)
        q2.append(q)
    s4, q4_ = [], []
    for i in range(2):
        s = lvl("lts", 4)
        nc.vector.tensor_add(out=s, in0=s2[2 * i], in1=s2[2 * i + 1])
        s4.append(s)
        q = lvl("ltq", 4)
        nc.vector.tensor_add(out=q, in0=q2[2 * i], in1=q2[2 * i + 1])
        q4_.append(q)
    s_all = lvl("lts", 8)
    nc.vector.tensor_add(out=s_all, in0=s4[0], in1=s4[1])
    q_all = lvl("ltq", 8)
    nc.vector.tensor_add(out=q_all, in0=q4_[0], in1=q4_[1])

    ps_s = psum_pool.tile([1, 512], F32, tag=psum_tag, bufs=psum_bufs,
                          name="ps_s")
    ps_q = psum_pool.tile([1, 512], F32, tag=psum_tag, bufs=psum_bufs,
                          name="ps_q")
    mm(out=ps_s, lhsT=ones_col, rhs=s_all, start=True, stop=True)
    mm(out=ps_q, lhsT=ones_col, rhs=q_all, start=True, stop=True)

    m = pool.tile([1, 512], F32, tag="ln_m", bufs=1, name="ln_m")
    e2 = pool.tile([1, 512], F32, tag="ln_e2", bufs=1, name="ln_e2")
    nc.vector.tensor_scalar_mul(out=m, in0=ps_s, scalar1=1.0 / D)
    nc.vector.tensor_scalar_mul(out=e2, in0=ps_q, scalar1=1.0 / D)
    msq = pool.tile([1, 512], F32, tag="ln_msq", bufs=1, name="ln_msq")
    nc.vector.tensor_mul(out=msq, in0=m, in1=m)
    var = pool.tile([1, 512], F32, tag="ln_var", bufs=1, name="ln_var")
    nc.vector.scalar_tensor_tensor(out=var, in0=e2, scalar=EPS, in1=msq,
                                   op0=Alu.add, op1=Alu.subtract)
    sd = pool.tile([1, 512], F32, tag="ln_sd", bufs=1, name="ln_sd")
    nc.scalar.activation(out=sd, in_=var, func=Act.Sqrt)
    a_row = pool.tile([1, 512], F32, tag="ln_a", bufs=1, name="ln_a")
    nc.vector.reciprocal_approx_fast(out=a_row, in_=sd)
    b_row = pool.tile([1, 512], F32, tag="ln_b", bufs=1, name="ln_b")
    nc.vector.scalar_tensor_tensor(out=b_row, in0=m, scalar=-1.0, in1=a_row,
                                   op0=Alu.mult, op1=Alu.mult)
    ac = pool.tile([1, 512], BF, tag="ln_ac", bufs=1, name="ln_ac")
    bc = pool.tile([1, 512], BF, tag="ln_bc", bufs=1, name="ln_bc")
    nc.vector.tensor_copy(out=ac, in_=a_row)
    nc.vector.tensor_copy(out=bc, in_=b_row)
    if bcast_mm is not None:
        # broadcast via K=1 matmul on the PE (avoids the gpsimd queue)
        ones_row, bpool, btag, bbufs = bcast_mm
        Ab = bpool.tile([128, 512], F32, tag=btag, bufs=bbufs, name="ln_Abp")
        Bb = bpool.tile([128, 512], F32, tag=btag, bufs=bbufs, name="ln_Bbp")
        mm(out=Ab, lhsT=ones_row, rhs=ac, start=True, stop=True)
        mm(out=Bb, lhsT=ones_row, rhs=bc, start=True, stop=True)
        return Ab, Bb
    Ab = pool.tile([128, 512], BF, tag="ln_Ab", bufs=2, name="ln_Ab")
    Bb = pool.tile([128, 512], BF, tag="ln_Bb", bufs=2, name="ln_Bb")
    nc.gpsimd.partition_broadcast(Ab, ac)
    nc.gpsimd.partition_broadcast(Bb, bc)
    return Ab, Bb


def _ln_apply(nc, pool, src_k, Ab, Bb, out_slice):
    """out = src*Ab + Bb (bf16)."""
    t1 = pool.tile([128, 512], BF, tag="ln_t1", bufs=2, name="ln_t1")
    nc.vector.tensor_mul(out=t1, in0=src_k, in1=Ab)
    nc.vector.tensor_add(out=out_slice, in0=t1, in1=Bb)


def _emit(nc, tc, t):
    mm = nc.tensor.matmul
    Alu = mybir.AluOpType
    Act = mybir.ActivationFunctionType

    xT_v = t["xT"].rearrange("(k p) t -> p k t", p=128)
    xres_v = t["xresT"].rearrange("(k p) t -> p k t", p=128)
    w1_v = t["w1"].rearrange("(k p) e -> p k e", p=128)
    w2_v = t["w2"].rearrange("(k p) e -> p k e", p=128)
    outT_v = t["outT"]

    # ---------------- persistent pools ----------------
    dram = tc.alloc_tile_pool(name="dram", bufs=1, space="DRAM")
    rs_in = [dram.tile([TP, D, 256], BF, name=f"rsi{c}") for c in range(NCH)]
    rs_out = [dram.tile([D, 256], BF, name=f"rso{c}") for c in range(NCH)]

    consts = tc.alloc_tile_pool(name="consts", bufs=1)
    ones_col = consts.tile([128, 1], BF)
    nc.vector.memset(ones_col, 1.0)
    ones_row = consts.tile([1, 128], BF)
    nc.vector.memset(ones_row, 1.0)

    bq_sb = consts.tile([128, KHE], F32)
    bk_sb = consts.tile([128, KHE], F32)
    bo2_sb = consts.tile([128, KD], F32)
    b2_sb = consts.tile([128, KD], F32)
    b1_sb = consts.tile([128, KFF], F32)
    for name, dst in (("bq", bq_sb), ("bk", bk_sb),
                      ("bo2", bo2_sb), ("b2", b2_sb)):
        nc.sync.dma_start(out=dst, in_=t[name].rearrange("(k p) -> p k", p=128))
    nc.sync.dma_start(out=b1_sb, in_=t["b1f"].rearrange("(k p) -> p k", p=128))
    # v bias broadcast over all partitions: [128, LHE]
    bvb = consts.tile([128, LHE], BF)
    bv_row = consts.tile([1, LHE], BF)
    nc.gpsimd.dma_start(out=bv_row,
                        in_=t["bv"].rearrange("(o e) -> o e", o=1))
    nc.gpsimd.partition_broadcast(bvb, bv_row)

    wlate = tc.alloc_tile_pool(name="wlate", bufs=1)
    wo_sb = wlate.tile([128, KHE, D], BF, tag="wo")
    nc.scalar.dma_start(out=wo_sb,
                        in_=t["wo"].rearrange("(k p) e -> p k e", p=128))

    # ================= Stage A: LN1 + QKV + attention + Wo + RS ============
    with tc.tile_pool(name="abc", bufs=1) as ab:
        kT = ab.tile([128, LH // 2, T], BF, tag="kT")
        qT = ab.tile([128, LH // 2, T], BF, tag="qT")
        vS = ab.tile([128, NST, LH * 128], BF, tag="vS")
        nc.vector.memset(vS, 0.0)
        for h in range(LH):
            nc.vector.memset(vS[:, :, h * 128 + 64:h * 128 + 65], 1.0)
        wq_sb = ab.tile([128, KD, LHE], BF, tag="wq")
        wk_sb = ab.tile([128, KD, LHE], BF, tag="wk")
        wv_sb = ab.tile([128, KD, LHE], BF, tag="wv")

        def load_qkv_weights():
            for eng, srct, dst in ((nc.scalar, t["wq"], wq_sb),
                                   (nc.scalar, t["wk"], wk_sb),
                                   (nc.sync, t["wv"], wv_sb)):
                eng.dma_start(out=dst,
                              in_=srct.rearrange("(k p) e -> p k e", p=128))

        with tc.tile_pool(name="projpsum", bufs=2, space="PSUM") as pproj, \
             tc.tile_pool(name="scpsum", bufs=OA_LAG, space="PSUM") as psc, \
             tc.tile_pool(name="oapsum", bufs=2, space="PSUM") as poa:

            def ln1(ci):
                xf = ab.tile([128, KD, 512], F32, tag="xf", bufs=2, name="xf")
                for k in range(KD):
                    eng = nc.sync if k % 2 == 0 else nc.scalar
                    eng.dma_start(out=xf[:, k:k + 1, :],
                                  in_=xT_v[:, k:k + 1,
                                           ci * 512:ci * 512 + 512])
                Ab, Bb = _ln_stats(nc, ab, pproj, "ps_proj", xf, ones_col)
                hT = ab.tile([128, KD, 512], BF, tag="hT", bufs=2, name="hT")
                for k in range(KD):
                    _ln_apply(nc, ab, xf[:, k, :], Ab, Bb, hT[:, k, :])
                return hT

            def proj_units(ci, hT):
                c0 = ci * 512
                units = []
                for w_sb, dst, bias in ((wk_sb, kT, bk_sb), (wq_sb, qT, bq_sb)):
                    for et in range(LH // 2):
                        def u(w_sb=w_sb, dst=dst, bias=bias, et=et, hT=hT,
                              c0=c0):
                            ps = pproj.tile([128, 512], F32, tag="ps_proj",
                                            bufs=2, name="ps_proj")
                            for k in range(KD):
                                mm(out=ps,
                                   lhsT=w_sb[:, k, et * 128:(et + 1) * 128],
                                   rhs=hT[:, k, :],
                                   start=(k == 0), stop=(k == KD - 1))
                            nc.vector.tensor_scalar(
                                out=dst[:, et, c0:c0 + 512], in0=ps,
                                scalar1=bias[:, et:et + 1], scalar2=None,
                                op0=Alu.add)
                        units.append(u)
                for sti in range(4):
                    st = ci * 4 + sti
                    def u(sti=sti, st=st, hT=hT):
                        ps = pproj.tile([128, LHE], F32, tag="ps_proj", bufs=2,
                                        name="ps_v")
                        for k in range(KD):
                            mm(out=ps,
                               lhsT=hT[:, k, sti * 128:sti * 128 + 128],
                               rhs=wv_sb[:, k, :],
                               start=(k == 0), stop=(k == KD - 1))
                        nc.vector.tensor_add(
                            out=vS[:, st, :].rearrange("p (h e) -> p h e",
                                                       h=LH)[:, :, 0:64],
                            in0=ps.rearrange("p (h e) -> p h e", e=64),
                            in1=bvb.rearrange("p (h e) -> p h e", e=64))
                    units.append(u)
                return units

            def attention(ci, oT, pending):
                c0 = ci * 512
                nb = 4 * (ci + 1)
                total_steps = (LH // 2) * (nb + OA_LAG)
                spacing = (max(1, total_steps // len(pending))
                           if pending else 0)
                stepctr = 0
                for hp in range(LH // 2):
                    po = [poa.tile([128, 512], F32, tag="po", bufs=2,
                                   name="po")
                          for _ in range(2)]
                    exs = [None] * nb

                    def scores(sb):
                        s0 = sb * 128
                        ps2 = psc.tile([128, 2, 512], F32, tag="ps_sc",
                                       bufs=OA_LAG, name="ps_sc")
                        for hi in range(2):
                            mm(out=ps2[:, hi, :],
                               lhsT=kT[hi * 64:hi * 64 + 64, hp, s0:s0 + 128],
                               rhs=qT[hi * 64:hi * 64 + 64, hp, c0:c0 + 512],
                               start=True, stop=True)
                        ex = ab.tile([128, 2, 512], BF, tag="ex",
                                     bufs=OA_LAG + 1, name="ex")
                        nc.scalar.activation(out=ex, in_=ps2, func=Act.Exp)
                        midx = sb - 4 * ci
                        if midx >= 0:
                            for hi in range(2):
                                nc.gpsimd.affine_select(
                                    out=ex[:, hi, :], in_=ex[:, hi, :],
                                    compare_op=Alu.is_ge, fill=0.0,
                                    base=-(midx * 128), channel_multiplier=-1,
                                    pattern=[[1, 512]])
                        exs[sb] = ex

                    def oacc(sb):
                        for hi in range(2):
                            h_loc = hp * 2 + hi
                            mm(out=po[hi],
                               lhsT=vS[:, sb,
                                       h_loc * 128:h_loc * 128 + 128],
                               rhs=exs[sb][:, hi, :],
                               start=(sb == 0), stop=(sb == nb - 1))

                    for step in range(nb + OA_LAG):
                        if step < nb:
                            scores(step)
                        if step >= OA_LAG:
                            oacc(step - OA_LAG)
                        stepctr += 1
                        if pending and stepctr % spacing == 0:
                            pending.pop(0)()

                    for hi in range(2):
                        h_loc = hp * 2 + hi
                        dnr = ab.tile([1, 512], F32, tag="dnr", bufs=1,
                                      name="dnr")
                        nc.vector.tensor_copy(out=dnr, in_=po[hi][64:65, :])
                        rcp = ab.tile([1, 512], F32, tag="rcp", bufs=2,
                                      name="rcp")
                        nc.vector.reciprocal_approx_fast(out=rcp, in_=dnr)
                        bc = ab.tile([64, 512], F32, tag="bc", bufs=1,
                                     name="bc")
                        nc.gpsimd.partition_broadcast(bc, rcp)
                        nc.vector.tensor_mul(
                            out=oT[hi * 64:hi * 64 + 64, hp, :],
                            in0=po[hi][0:64, :], in1=bc)
                while pending:
                    pending.pop(0)()

            def wo_units(ci, oT):
                rsv = rs_in[ci].rearrange("j (k p) t -> j k p t", p=128)
                units = []
                for dt in range(KD):
                    def u(dt=dt, oT=oT, rsv=rsv, last=(ci == NCH - 1)):
                        ps = pproj.tile([128, 512], F32, tag="ps_proj",
                                        bufs=2, name="ps_wo")
                        for k in range(KHE):
                            mm(out=ps,
                               lhsT=wo_sb[:, k, dt * 128:(dt + 1) * 128],
                               rhs=oT[:, k, :],
                               start=(k == 0), stop=(k == KHE - 1))
                        stg = ab.tile([128, 512], BF, tag="stg1", bufs=2,
                                      name="stg1")
                        if last:
                            nc.scalar.activation(
                                out=stg, in_=ps, func=Act.Identity,
                                bias=bo2_sb[:, dt:dt + 1])
                        else:
                            nc.vector.tensor_scalar(
                                out=stg, in0=ps, scalar1=bo2_sb[:, dt:dt + 1],
                                scalar2=None, op0=Alu.add)
                        for j in range(TP):
                            nc.sync.dma_start(
                                out=rsv[j, dt, :, :],
                                in_=stg[:, j * 256:(j + 1) * 256])
                    units.append(u)
                return units

            def rs_issue(ci):
                nc.gpsimd.collective_compute(
                    "ReduceScatter", Alu.add, replica_groups=PAIRS,
                    ins=[rs_in[ci].opt()], outs=[rs_out[ci].opt()])

            hTs = {0: ln1(0)}
            load_qkv_weights()
            for u in proj_units(0, hTs[0]):
                u()
            oTs = {}
            for ci in range(NCH):
                pending = []
                if ci >= 1:
                    pending += wo_units(ci - 1, oTs[ci - 1])
                if ci + 1 < NCH:
                    hTs[ci + 1] = ln1(ci + 1)
                    pending += proj_units(ci + 1, hTs[ci + 1])
                oTs[ci] = ab.tile([128, KHE, 512], BF, tag="oT", bufs=2,
                                  name="oT")
                attention(ci, oTs[ci], pending)
                if ci >= 1:
                    rs_issue(ci - 1)
            for u in wo_units(NCH - 1, oTs[NCH - 1]):
                u()
            rs_issue(NCH - 1)

    # ================= Stage B: residual + LN2 + FFN (own T/2 rows) ========
    with tc.tile_pool(name="de", bufs=1) as de, \
         tc.tile_pool(name="upsum", bufs=3, space="PSUM") as pu, \
         tc.tile_pool(name="fpsum", bufs=2, space="PSUM") as pf:

        w2_sb = de.tile([128, KFF, D], BF, tag="w2t", bufs=1, name="w2t")

        def prep(lc):
            c0 = lc * 512
            # residual: xmid = xres + rs_out  (bf16 residual stream)
            xr = de.tile([128, KD, 512], BF, tag="xr", bufs=1, name="xr")
            nc.sync.dma_start(out=xr[:, 0:4, :],
                              in_=xres_v[:, 0:4, c0:c0 + 512])
            nc.scalar.dma_start(out=xr[:, 4:8, :],
                                in_=xres_v[:, 4:8, c0:c0 + 512])
            arr = de.tile([128, KD, 2, 256], BF, tag="arr", bufs=1, name="arr")
            rsv0 = rs_out[2 * lc].rearrange("(k p) t -> p k t", p=128)
            rsv1 = rs_out[2 * lc + 1].rearrange("(k p) t -> p k t", p=128)
            nc.gpsimd.dma_start(out=arr[:, :, 0, :], in_=rsv0)
            nc.sync.dma_start(out=arr[:, 0:4, 1, :], in_=rsv1[:, 0:4, :])
            nc.scalar.dma_start(out=arr[:, 4:8, 1, :], in_=rsv1[:, 4:8, :])
            xmid = de.tile([128, KD, 512], BF, tag="xmid", bufs=2,
                           name="xmid")
            for k in range(KD):
                nc.vector.tensor_add(
                    out=xmid[:, k, :], in0=xr[:, k, :],
                    in1=arr[:, k, :, :].rearrange("p j t -> p (j t)"))
            # LN2 (gains folded into W1/b1f on host)
            Ab2, Bb2 = _ln_stats(nc, de, pu, "ps_u", xmid, ones_col,
                                 psum_bufs=3,
                                 bcast_mm=(ones_row, pf, "ps_f", 2))
            h2 = de.tile([128, KD, 512], BF, tag="h2", bufs=1, name="h2")
            for k in range(KD):
                _ln_apply(nc, de, xmid[:, k, :], Ab2, Bb2, h2[:, k, :])
            return xmid, h2

        def fc1(h2):
            # FFN up: u = relu(h2 @ W1 + b1f)   (relu+bias on DVE)
            u = de.tile([128, KFF, 512], BF, tag="u", bufs=1, name="u")
            for q16 in range(16):
                w1t = de.tile([128, KD, 256], BF, tag="w1t", bufs=2,
                              name="w1t")
                nc.sync.dma_start(out=w1t,
                                  in_=w1_v[:, :, q16 * 256:(q16 + 1) * 256])
                for fi in range(2):
                    fft = q16 * 2 + fi
                    ps = pu.tile([128, 512], F32, tag="ps_u", bufs=3,
                                 name="ps_u")
                    for k in range(KD):
                        mm(out=ps,
                           lhsT=w1t[:, k, fi * 128:fi * 128 + 128],
                           rhs=h2[:, k, :],
                           start=(k == 0), stop=(k == KD - 1))
                    nc.vector.tensor_scalar(
                        out=u[:, fft, :], in0=ps,
                        scalar1=b1_sb[:, fft:fft + 1], scalar2=0.0,
                        op0=Alu.add, op1=Alu.max)
            return u

        def fc2(lc, u, xmid):
            c0 = lc * 512
            # FFN down + bias + residual -> store
            for dt in range(KD):
                ps = pf.tile([128, 512], F32, tag="ps_f", bufs=2, name="ps_f")
                for k2 in range(KFF):
                    mm(out=ps,
                       lhsT=w2_sb[:, k2, dt * 128:(dt + 1) * 128],
                       rhs=u[:, k2, :],
                       start=(k2 == 0), stop=(k2 == KFF - 1))
                o_f = de.tile([128, 512], F32, tag="o_f", bufs=2, name="o_f")
                nc.vector.scalar_tensor_tensor(
                    out=o_f, in0=ps, scalar=b2_sb[:, dt:dt + 1],
                    in1=xmid[:, dt, :], op0=Alu.add, op1=Alu.add)
                nc.sync.dma_start(
                    out=outT_v[dt * 128:(dt + 1) * 128, c0:c0 + 512],
                    in_=o_f)

        xmid0, h20 = prep(0)
        u0 = fc1(h20)
        for q4 in range(4):
            nc.gpsimd.dma_start(
                out=w2_sb[:, q4 * 8:(q4 + 1) * 8, :],
                in_=w2_v[:, q4 * 8:(q4 + 1) * 8, :])
        xmid1, h21 = prep(1)
        fc2(0, u0, xmid0)
        u1 = fc1(h21)
        fc2(1, u1, xmid1)

    wlate.release()
    consts.release()
    dram.release()


def _build():
    nc = bacc.Bacc("TRN2", target_bir_lowering=False, debug=False,
                   num_devices=NCORES)

    tensors = {}
    tensors["xT"] = nc.dram_tensor("xT", [D, T], F32, kind="ExternalInput").ap()
    tensors["xresT"] = nc.dram_tensor("xresT", [D, LT], BF,
                                      kind="ExternalInput").ap()
    for name, shape, dt in (
        ("wq", [D, LHE], BF), ("wk", [D, LHE], BF), ("wv", [D, LHE], BF),
        ("wo", [LHE, D], BF), ("w1", [D, FF], BF), ("w2", [FF, D], BF),
        ("bq", [LHE], F32), ("bk", [LHE], F32), ("bv", [LHE], F32),
        ("b1f", [FF], F32), ("bo2", [D], F32), ("b2", [D], F32),
    ):
        tensors[name] = nc.dram_tensor(name, shape, dt,
                                       kind="ExternalInput").ap()
    tensors["outT"] = nc.dram_tensor("out", [D, LT], F32,
                                     kind="ExternalOutput").ap()

    with tile.TileContext(nc, num_cores=NCORES) as tc:
        _emit(nc, tc, tensors)

    nc.compile()
    return nc


_NC_CACHE = None


def _get_nc():
    global _NC_CACHE
    if _NC_CACHE is None:
        _NC_CACHE = _build()
    return _NC_CACHE


def _shard_inputs(x, Wq, Wk, Wv, Wo, bo, W1, b1, W2, b2, g1, be1, g2, be2):
    """Build the 8 per-core input maps (LN gains folded into weights)."""
    bf = lambda a: np.ascontiguousarray(a).astype(BF16NP)
    f32 = lambda a: np.ascontiguousarray(a, dtype=np.float32)

    x = np.asarray(x, dtype=np.float32)
    Wq = np.asarray(Wq, dtype=np.float32)
    Wk = np.asarray(Wk, dtype=np.float32)
    Wv = np.asarray(Wv, dtype=np.float32)
    Wo = np.asarray(Wo, dtype=np.float32)
    W1 = np.asarray(W1, dtype=np.float32)
    W2 = np.asarray(W2, dtype=np.float32)
    g1 = np.asarray(g1, dtype=np.float32)
    be1 = np.asarray(be1, dtype=np.float32)
    g2 = np.asarray(g2, dtype=np.float32)
    be2 = np.asarray(be2, dtype=np.float32)
    b1 = np.asarray(b1, dtype=np.float32)

    scale = float(HS) ** -0.5
    # fold g1 into QKV weights, be1 into QKV biases; fold the score scale
    # into Wq/bq.  Per-head [H, D, HS] -> concat heads -> [D, H*HS].
    wq_f = (g1[None, :, None] * Wq).transpose(1, 0, 2).reshape(D, D) * scale
    wk_f = (g1[None, :, None] * Wk).transpose(1, 0, 2).reshape(D, D)
    wv_f = (g1[None, :, None] * Wv).transpose(1, 0, 2).reshape(D, D)
    bq_f = np.einsum("d,hde->he", be1, Wq).reshape(D) * scale
    bk_f = np.einsum("d,hde->he", be1, Wk).reshape(D)
    bv_f = np.einsum("d,hde->he", be1, Wv).reshape(D)
    # fold g2/be2 into W1/b1
    w1_f = g2[:, None] * W1
    b1_f = b1 + be2 @ W1

    in_maps = []
    for c in range(NCORES):
        b, half = divmod(c, TP)
        hes = slice(half * LHE, (half + 1) * LHE)
        xt = x[b].T
        xres = np.concatenate(
            [xt[:, ci * 512 + half * 256: ci * 512 + half * 256 + 256]
             for ci in range(NCH)], axis=1)
        in_maps.append({
            "xT": f32(xt),
            "xresT": bf(xres),
            "wq": bf(wq_f[:, hes]), "wk": bf(wk_f[:, hes]),
            "wv": bf(wv_f[:, hes]),
            "bq": f32(bq_f[hes]), "bk": f32(bk_f[hes]), "bv": f32(bv_f[hes]),
            "wo": bf(Wo[hes, :]),
            "bo2": f32(np.asarray(bo, dtype=np.float32) / TP),
            "w1": bf(w1_f), "b1f": f32(b1_f),
            "w2": bf(W2), "b2": f32(np.asarray(b2, dtype=np.float32)),
        })
    return in_maps


def kernel(x, Wq, Wk, Wv, Wo, bo, W1, b1, W2, b2, g1, be1, g2, be2,
           _trace=False):
    nc = _get_nc()
    in_maps = _shard_inputs(x, Wq, Wk, Wv, Wo, bo, W1, b1, W2, b2,
                            g1, be1, g2, be2)
    res = run_bass_kernel_spmd(nc, in_maps, list(range(NCORES)),
                               trace=_trace)
    out = np.empty((B, T, D), dtype=np.float32)
    for b in range(B):
        for half in range(TP):
            o = res.results[TP * b + half]["out"]  # [D, LT]
            for ci in range(NCH):
                t0 = ci * 512 + half * 256
                out[b, t0:t0 + 256, :] = o[:, ci * 256:(ci + 1) * 256].T
    if _trace:
        kernel.last_exec_time_ns = res.exec_time_ns
        kernel.last_results = res
    return out


# revision 29
# speedup vs baseline: 1.0184x; 1.0169x over previous
"""Trainium2 Bass kernel for a pre-LN transformer block (B=4, T=2048, D=1024,
H=16, HS=64, FF=4096, causal attention).

Sharding: data-parallel over batches x 2-way tensor-parallel attention
(8 heads/core over all T) -> pair ReduceScatter of the attention-output
projection over the sequence dim -> sequence-parallel FFN (full FF width,
T/2 rows per core).  No AllReduce anywhere; each core emits the final
output for its own T/2 rows.

Core c (0..7): batch b = c//2, half = c%2.  half h owns t-slices
[ci*512 + h*256, ci*512 + h*256 + 256) for ci in 0..3.

Layout: activations feature-major (d on partitions, t on free dim).
LayerNorm gains are folded into the weights on the host; LN on-chip is
just (x - mu) * inv_sigma with stats from DVE adder trees + one-column
matmuls that share the projection PSUM slots.  Attention is
phase-separated per (head-pair, chunk): score matmuls run a few steps
ahead of the o-accum matmuls with exp ([128,2,512] double-bank ACT ops)
in between, so the PE never stalls behind the scalar engine.
"""

import numpy as np
import ml_dtypes

import concourse.bacc as bacc
import concourse.bass as bass
import concourse.mybir as mybir
import concourse.tile as tile
from concourse.bass_utils import run_bass_kernel_spmd

BF16NP = ml_dtypes.bfloat16

B, T, D, H, HS, FF = 4, 2048, 1024, 16, 64, 4096
EPS = 1e-5
NCORES = 8
TP = 2
LH = H // TP          # 8 local heads
LHE = LH * HS         # 512 local head-embed width
LT = T // TP          # 1024 local rows (FFN/output)
KD = D // 128         # 8 d k-tiles
KHE = LHE // 128      # 4 he k-tiles
KFF = FF // 128       # 32 ff tiles
NCH = T // 512        # 4 t-chunks of 512
NST = T // 128        # 16 s-tiles of 128
PAIRS = [[0, 1], [2, 3], [4, 5], [6, 7]]
OA_LAG = 2            # psc tiles in flight between scores and o-accum

F32 = mybir.dt.float32
BF = mybir.dt.bfloat16


def _ln_stats(nc, pool, psum_pool, psum_tag, src, ones_col, psum_bufs=2,
              sq_on_act=False, bcast_mm=None):
    """src: [128, KD, 512] AP.  Returns (Ab, Bb) [128,512] bf16 broadcast
    tiles so that xn = src*Ab + Bb.  Stats via bf16 DVE adder trees plus
    two one-column matmuls that borrow slots from psum_pool/psum_tag.
    bcast_mm=(ones_row, pool, tag, bufs) broadcasts via PE matmul into
    PSUM instead of gpsimd partition_broadcast."""
    mm = nc.tensor.matmul
    Alu = mybir.AluOpType
    Act = mybir.ActivationFunctionType

    def lvl(tg, n):
        return pool.tile([128, 512], BF, tag=f"{tg}{n}", bufs=2, name=tg)

    s2, q2 = [], []
    for i in range(4):
        s = lvl("lts", 2)
        nc.vector.tensor_add(out=s, in0=src[:, 2 * i, :],
                             in1=src[:, 2 * i + 1, :])
        s2.append(s)
        sqa = pool.tile([128, 512], BF, tag="ln_sq", bufs=2, name="ln_sq")
        sqb = pool.tile([128, 512], BF, tag="ln_sq", bufs=2, name="ln_sq")
        if sq_on_act:
            nc.scalar.square(out=sqa, in_=src[:, 2 * i, :])
            nc.scalar.square(out=sqb, in_=src[:, 2 * i + 1, :])
        else:
            nc.vector.tensor_mul(out=sqa, in0=src[:, 2 * i, :],
                                 in1=src[:, 2 * i, :])
            nc.vector.tensor_mul(out=sqb, in0=src[:, 2 * i + 1, :],
                                 in1=src[:, 2 * i + 1, :])
        q = lvl("ltq", 2)
        nc.vector.tensor_add(out=q, in0=sqa, in1=sqb)
        q2.append(q)
    s4, q4_ = [], []
    for i in range(2):
        s = lvl("lts", 4)
        nc.vector.tensor_add(out=s, in0=s2[2 * i], in1=s2[2 * i + 1])
        s4.append(s)
        q = lvl("ltq", 4)
        nc.vector.tensor_add(out=q, in0=q2[2 * i], in1=q2[2 * i + 1])
        q4_.append(q)
    s_all = lvl("lts", 8)
    nc.vector.tensor_add(out=s_all, in0=s4[0], in1=s4[1])
    q_all = lvl("ltq", 8)
    nc.vector.tensor_add(out=q_all, in0=q4_[0], in1=q4_[1])

    ps_s = psum_pool.tile([1, 512], F32, tag=psum_tag, bufs=psum_bufs,
                          name="ps_s")
    ps_q = psum_pool.tile([1, 512], F32, tag=psum_tag, bufs=psum_bufs,
                          name="ps_q")
    mm(out=ps_s, lhsT=ones_col, rhs=s_all, start=True, stop=True)
    mm(out=ps_q, lhsT=ones_col, rhs=q_all, start=True, stop=True)

    m = pool.tile([1, 512], F32, tag="ln_m", bufs=1, name="ln_m")
    e2 = pool.tile([1, 512], F32, tag="ln_e2", bufs=1, name="ln_e2")
    nc.vector.tensor_scalar_mul(out=m, in0=ps_s, scalar1=1.0 / D)
    nc.vector.tensor_scalar_mul(out=e2, in0=ps_q, scalar1=1.0 / D)
    msq = pool.tile([1, 512], F32, tag="ln_msq", bufs=1, name="ln_msq")
    nc.vector.tensor_mul(out=msq, in0=m, in1=m)
    var = pool.tile([1, 512], F32, tag="ln_var", bufs=1, name="ln_var")
    nc.vector.scalar_tensor_tensor(out=var, in0=e2, scalar=EPS, in1=msq,
                                   op0=Alu.add, op1=Alu.subtract)
    sd = pool.tile([1, 512], F32, tag="ln_sd", bufs=1, name="ln_sd")
    nc.scalar.activation(out=sd, in_=var, func=Act.Sqrt)
    a_row = pool.tile([1, 512], F32, tag="ln_a", bufs=1, name="ln_a")
    nc.vector.reciprocal_approx_fast(out=a_row, in_=sd)
    b_row = pool.tile([1, 512], F32, tag="ln_b", bufs=1, name="ln_b")
    nc.vector.scalar_tensor_tensor(out=b_row, in0=m, scalar=-1.0, in1=a_row,
                                   op0=Alu.mult, op1=Alu.mult)
    ac = pool.tile([1, 512], BF, tag="ln_ac", bufs=1, name="ln_ac")
    bc = pool.tile([1, 512], BF, tag="ln_bc", bufs=1, name="ln_bc")
    nc.vector.tensor_copy(out=ac, in_=a_row)
    nc.vector.tensor_copy(out=bc, in_=b_row)
    if bcast_mm is not None:
        # broadcast via K=1 matmul on the PE (avoids the gpsimd queue)
        ones_row, bpool, btag, bbufs = bcast_mm
        Ab = bpool.tile([128, 512], F32, tag=btag, bufs=bbufs, name="ln_Abp")
        Bb = bpool.tile([128, 512], F32, tag=btag, bufs=bbufs, name="ln_Bbp")
        mm(out=Ab, lhsT=ones_row, rhs=ac, start=True, stop=True)
        mm(out=Bb, lhsT=ones_row, rhs=bc, start=True, stop=True)
        return Ab, Bb
    Ab = pool.tile([128, 512], BF, tag="ln_Ab", bufs=2, name="ln_Ab")
    Bb = pool.tile([128, 512], BF, tag="ln_Bb", bufs=2, name="ln_Bb")
    nc.gpsimd.partition_broadcast(Ab, ac)
    nc.gpsimd.partition_broadcast(Bb, bc)
    return Ab, Bb


def _ln_apply(nc, pool, src_k, Ab, Bb, out_slice):
    """out = src*Ab + Bb (bf16)."""
    t1 = pool.tile([128, 512], BF, tag="ln_t1", bufs=2, name="ln_t1")
    nc.vector.tensor_mul(out=t1, in0=src_k, in1=Ab)
    nc.vector.tensor_add(out=out_slice, in0=t1, in1=Bb)


def _emit(nc, tc, t):
    mm = nc.tensor.matmul
    Alu = mybir.AluOpType
    Act = mybir.ActivationFunctionType

    outT_v = t["outT"]

    # ---------------- persistent pools ----------------
    dram = tc.alloc_tile_pool(name="dram", bufs=1, space="DRAM")
    rs_in = [dram.tile([TP, 128, KD, 256], BF, name=f"rsi{c}")
             for c in range(NCH)]
    rs_out = [dram.tile([128, KD, 256], BF, name=f"rso{c}")
              for c in range(NCH)]

    consts = tc.alloc_tile_pool(name="consts", bufs=1)
    ones_col = consts.tile([128, 1], BF)
    nc.vector.memset(ones_col, 1.0)
    ones_row = consts.tile([1, 128], BF)
    nc.vector.memset(ones_row, 1.0)

    bq_sb = consts.tile([128, KHE], F32)
    bk_sb = consts.tile([128, KHE], F32)
    bo2_sb = consts.tile([128, KD], F32)
    b2_sb = consts.tile([128, KD], F32)
    b1_sb = consts.tile([128, KFF], F32)
    for name, dst in (("bq", bq_sb), ("bk", bk_sb),
                      ("bo2", bo2_sb), ("b2", b2_sb)):
        nc.sync.dma_start(out=dst, in_=t[name].rearrange("(k p) -> p k", p=128))
    nc.sync.dma_start(out=b1_sb, in_=t["b1f"].rearrange("(k p) -> p k", p=128))
    # v bias broadcast over all partitions: [128, LHE]
    bvb = consts.tile([128, LHE], BF)
    bv_row = consts.tile([1, LHE], BF)
    nc.gpsimd.dma_start(out=bv_row,
                        in_=t["bv"].rearrange("(o e) -> o e", o=1))
    nc.gpsimd.partition_broadcast(bvb, bv_row)

    wlate = tc.alloc_tile_pool(name="wlate", bufs=1)
    wo_sb = wlate.tile([128, KHE, D], BF, tag="wo")
    nc.scalar.dma_start(out=wo_sb, in_=t["wo"])

    # ================= Stage A: LN1 + QKV + attention + Wo + RS ============
    with tc.tile_pool(name="abc", bufs=1) as ab:
        kT = ab.tile([128, LH // 2, T], BF, tag="kT")
        qT = ab.tile([128, LH // 2, T], BF, tag="qT")
        vS = ab.tile([128, NST, LH * 128], BF, tag="vS")
        nc.vector.memset(vS, 0.0)
        for h in range(LH):
            nc.vector.memset(vS[:, :, h * 128 + 64:h * 128 + 65], 1.0)
        wq_sb = ab.tile([128, KD, LHE], BF, tag="wq")
        wk_sb = ab.tile([128, KD, LHE], BF, tag="wk")
        wv_sb = ab.tile([128, KD, LHE], BF, tag="wv")

        def load_qkv_weights():
            for eng, srct, dst in ((nc.scalar, t["wq"], wq_sb),
                                   (nc.scalar, t["wk"], wk_sb),
                                   (nc.sync, t["wv"], wv_sb)):
                eng.dma_start(out=dst, in_=srct)

        with tc.tile_pool(name="projpsum", bufs=2, space="PSUM") as pproj, \
             tc.tile_pool(name="scpsum", bufs=OA_LAG, space="PSUM") as psc, \
             tc.tile_pool(name="oapsum", bufs=2, space="PSUM") as poa:

            def ln1(ci):
                xf = ab.tile([128, KD, 512], F32, tag="xf", bufs=2, name="xf")
                for k in range(KD):
                    eng = nc.sync if k % 2 == 0 else nc.scalar
                    eng.dma_start(out=xf[:, k:k + 1, :],
                                  in_=t["xT"][ci * 128:ci * 128 + 128,
                                              k:k + 1, :])
                Ab, Bb = _ln_stats(nc, ab, pproj, "ps_proj", xf, ones_col)
                hT = ab.tile([128, KD, 512], BF, tag="hT", bufs=2, name="hT")
                for k in range(KD):
                    _ln_apply(nc, ab, xf[:, k, :], Ab, Bb, hT[:, k, :])
                return hT

            def proj_units(ci, hT):
                c0 = ci * 512
                units = []
                for w_sb, dst, bias in ((wk_sb, kT, bk_sb), (wq_sb, qT, bq_sb)):
                    for et in range(LH // 2):
                        def u(w_sb=w_sb, dst=dst, bias=bias, et=et, hT=hT,
                              c0=c0):
                            ps = pproj.tile([128, 512], F32, tag="ps_proj",
                                            bufs=2, name="ps_proj")
                            for k in range(KD):
                                mm(out=ps,
                                   lhsT=w_sb[:, k, et * 128:(et + 1) * 128],
                                   rhs=hT[:, k, :],
                                   start=(k == 0), stop=(k == KD - 1))
                            nc.vector.tensor_scalar(
                                out=dst[:, et, c0:c0 + 512], in0=ps,
                                scalar1=bias[:, et:et + 1], scalar2=None,
                                op0=Alu.add)
                        units.append(u)
                for sti in range(4):
                    st = ci * 4 + sti
                    def u(sti=sti, st=st, hT=hT):
                        ps = pproj.tile([128, LHE], F32, tag="ps_proj", bufs=2,
                                        name="ps_v")
                        for k in range(KD):
                            mm(out=ps,
                               lhsT=hT[:, k, sti * 128:sti * 128 + 128],
                               rhs=wv_sb[:, k, :],
                               start=(k == 0), stop=(k == KD - 1))
                        nc.vector.tensor_add(
                            out=vS[:, st, :].rearrange("p (h e) -> p h e",
                                                       h=LH)[:, :, 0:64],
                            in0=ps.rearrange("p (h e) -> p h e", e=64),
                            in1=bvb.rearrange("p (h e) -> p h e", e=64))
                    units.append(u)
                return units

            def attention(ci, oT, pending):
                c0 = ci * 512
                nb = 4 * (ci + 1)
                total_steps = (LH // 2) * (nb + OA_LAG)
                spacing = (max(1, total_steps // len(pending))
                           if pending else 0)
                stepctr = 0
                for hp in range(LH // 2):
                    po = [poa.tile([128, 512], F32, tag="po", bufs=2,
                                   name="po")
                          for _ in range(2)]
                    exs = [None] * nb

                    def scores(sb):
                        s0 = sb * 128
                        ps2 = psc.tile([128, 2, 512], F32, tag="ps_sc",
                                       bufs=OA_LAG, name="ps_sc")
                        for hi in range(2):
                            mm(out=ps2[:, hi, :],
                               lhsT=kT[hi * 64:hi * 64 + 64, hp, s0:s0 + 128],
                               rhs=qT[hi * 64:hi * 64 + 64, hp, c0:c0 + 512],
                               start=True, stop=True)
                        ex = ab.tile([128, 2, 512], BF, tag="ex",
                                     bufs=OA_LAG + 1, name="ex")
                        nc.scalar.activation(out=ex, in_=ps2, func=Act.Exp)
                        midx = sb - 4 * ci
                        if midx >= 0:
                            for hi in range(2):
                                nc.gpsimd.affine_select(
                                    out=ex[:, hi, :], in_=ex[:, hi, :],
                                    compare_op=Alu.is_ge, fill=0.0,
                                    base=-(midx * 128), channel_multiplier=-1,
                                    pattern=[[1, 512]])
                        exs[sb] = ex

                    def oacc(sb):
                        for hi in range(2):
                            h_loc = hp * 2 + hi
                            mm(out=po[hi],
                               lhsT=vS[:, sb,
                                       h_loc * 128:h_loc * 128 + 128],
                               rhs=exs[sb][:, hi, :],
                               start=(sb == 0), stop=(sb == nb - 1))

                    for step in range(nb + OA_LAG):
                        if step < nb:
                            scores(step)
                        if step >= OA_LAG:
                            oacc(step - OA_LAG)
                        stepctr += 1
                        if pending and stepctr % spacing == 0:
                            pending.pop(0)()

                    for hi in range(2):
                        h_loc = hp * 2 + hi
                        dnr = ab.tile([1, 512], F32, tag="dnr", bufs=1,
                                      name="dnr")
                        nc.vector.tensor_copy(out=dnr, in_=po[hi][64:65, :])
                        rcp = ab.tile([1, 512], F32, tag="rcp", bufs=2,
                                      name="rcp")
                        nc.vector.reciprocal_approx_fast(out=rcp, in_=dnr)
                        bc = ab.tile([64, 512], F32, tag="bc", bufs=1,
                                     name="bc")
                        nc.gpsimd.partition_broadcast(bc, rcp)
                        nc.vector.tensor_mul(
                            out=oT[hi * 64:hi * 64 + 64, hp, :],
                            in0=po[hi][0:64, :], in1=bc)
                while pending:
                    pending.pop(0)()

            def wo_units(ci, oT):
                rsv = rs_in[ci]
                units = []
                for dt in range(KD):
                    def u(dt=dt, oT=oT, rsv=rsv, last=(ci == NCH - 1)):
                        ps = pproj.tile([128, 512], F32, tag="ps_proj",
                                        bufs=2, name="ps_wo")
                        for k in range(KHE):
                            mm(out=ps,
                               lhsT=wo_sb[:, k, dt * 128:(dt + 1) * 128],
                               rhs=oT[:, k, :],
                               start=(k == 0), stop=(k == KHE - 1))
                        stg = ab.tile([128, 512], BF, tag="stg1", bufs=2,
                                      name="stg1")
                        if last:
                            nc.scalar.activation(
                                out=stg, in_=ps, func=Act.Identity,
                                bias=bo2_sb[:, dt:dt + 1])
                        else:
                            nc.vector.tensor_scalar(
                                out=stg, in0=ps, scalar1=bo2_sb[:, dt:dt + 1],
                                scalar2=None, op0=Alu.add)
                        for j in range(TP):
                            nc.sync.dma_start(
                                out=rsv[j, :, dt, :],
                                in_=stg[:, j * 256:(j + 1) * 256])
                    units.append(u)
                return units

            def rs_issue(ci):
                nc.gpsimd.collective_compute(
                    "ReduceScatter", Alu.add, replica_groups=PAIRS,
                    ins=[rs_in[ci].opt()], outs=[rs_out[ci].opt()])

            hTs = {0: ln1(0)}
            load_qkv_weights()
            for u in proj_units(0, hTs[0]):
                u()
            oTs = {}
            for ci in range(NCH):
                pending = []
                if ci >= 1:
                    pending += wo_units(ci - 1, oTs[ci - 1])
                if ci + 1 < NCH:
                    hTs[ci + 1] = ln1(ci + 1)
                    pending += proj_units(ci + 1, hTs[ci + 1])
                oTs[ci] = ab.tile([128, KHE, 512], BF, tag="oT", bufs=2,
                                  name="oT")
                attention(ci, oTs[ci], pending)
                if ci >= 1:
                    rs_issue(ci - 1)
            for u in wo_units(NCH - 1, oTs[NCH - 1]):
                u()
            rs_issue(NCH - 1)

    # ================= Stage B: residual + LN2 + FFN (own T/2 rows) ========
    with tc.tile_pool(name="de", bufs=1) as de, \
         tc.tile_pool(name="upsum", bufs=3, space="PSUM") as pu, \
         tc.tile_pool(name="fpsum", bufs=2, space="PSUM") as pf:

        w2_sb = de.tile([128, KFF, D], BF, tag="w2t", bufs=1, name="w2t")

        def prep(lc):
            c0 = lc * 512
            # residual: xmid = xres + rs_out  (bf16 residual stream)
            xr = de.tile([128, KD, 512], BF, tag="xr", bufs=1, name="xr")
            xrs = t["xresT"][lc * 128:lc * 128 + 128, :, :]
            nc.sync.dma_start(out=xr[:, 0:4, :], in_=xrs[:, 0:4, :])
            nc.scalar.dma_start(out=xr[:, 4:8, :], in_=xrs[:, 4:8, :])
            arr = de.tile([128, KD, 2, 256], BF, tag="arr", bufs=1, name="arr")
            nc.gpsimd.dma_start(out=arr[:, :, 0, :], in_=rs_out[2 * lc])
            nc.sync.dma_start(out=arr[:, 0:4, 1, :],
                              in_=rs_out[2 * lc + 1][:, 0:4, :])
            nc.scalar.dma_start(out=arr[:, 4:8, 1, :],
                                in_=rs_out[2 * lc + 1][:, 4:8, :])
            xmid = de.tile([128, KD, 512], BF, tag="xmid", bufs=2,
                           name="xmid")
            for k in range(KD):
                nc.vector.tensor_add(
                    out=xmid[:, k, :], in0=xr[:, k, :],
                    in1=arr[:, k, :, :].rearrange("p j t -> p (j t)"))
            # LN2 (gains folded into W1/b1f on host)
            Ab2, Bb2 = _ln_stats(nc, de, pu, "ps_u", xmid, ones_col,
                                 psum_bufs=3,
                                 bcast_mm=(ones_row, pf, "ps_f", 2))
            h2 = de.tile([128, KD, 512], BF, tag="h2", bufs=1, name="h2")
            for k in range(KD):
                _ln_apply(nc, de, xmid[:, k, :], Ab2, Bb2, h2[:, k, :])
            return xmid, h2

        def fc1(h2):
            # FFN up: u = relu(h2 @ W1 + b1f)   (relu+bias on DVE)
            u = de.tile([128, KFF, 512], BF, tag="u", bufs=1, name="u")
            for q16 in range(16):
                w1t = de.tile([128, KD, 256], BF, tag="w1t", bufs=2,
                              name="w1t")
                nc.sync.dma_start(
                    out=w1t, in_=t["w1"][q16 * 128:(q16 + 1) * 128, :, :])
                for fi in range(2):
                    fft = q16 * 2 + fi
                    ps = pu.tile([128, 512], F32, tag="ps_u", bufs=3,
                                 name="ps_u")
                    for k in range(KD):
                        mm(out=ps,
                           lhsT=w1t[:, k, fi * 128:fi * 128 + 128],
                           rhs=h2[:, k, :],
                           start=(k == 0), stop=(k == KD - 1))
                    nc.vector.tensor_scalar(
                        out=u[:, fft, :], in0=ps,
                        scalar1=b1_sb[:, fft:fft + 1], scalar2=0.0,
                        op0=Alu.add, op1=Alu.max)
            return u

        def fc2(lc, u, xmid):
            c0 = lc * 512
            # FFN down + bias + residual -> store
            for dt in range(KD):
                ps = pf.tile([128, 512], F32, tag="ps_f", bufs=2, name="ps_f")
                for k2 in range(KFF):
                    mm(out=ps,
                       lhsT=w2_sb[:, k2, dt * 128:(dt + 1) * 128],
                       rhs=u[:, k2, :],
                       start=(k2 == 0), stop=(k2 == KFF - 1))
                o_f = de.tile([128, 512], F32, tag="o_f", bufs=2, name="o_f")
                nc.vector.scalar_tensor_tensor(
                    out=o_f, in0=ps, scalar=b2_sb[:, dt:dt + 1],
                    in1=xmid[:, dt, :], op0=Alu.add, op1=Alu.add)
                nc.sync.dma_start(
                    out=outT_v[dt * 128:(dt + 1) * 128, c0:c0 + 512],
                    in_=o_f)

        xmid0, h20 = prep(0)
        u0 = fc1(h20)
        for q4 in range(4):
            nc.gpsimd.dma_start(
                out=w2_sb[:, q4 * 8:(q4 + 1) * 8, :],
                in_=t["w2"][q4 * 128:(q4 + 1) * 128, :, :])
        xmid1, h21 = prep(1)
        fc2(0, u0, xmid0)
        u1 = fc1(h21)
        fc2(1, u1, xmid1)

    wlate.release()
    consts.release()
    dram.release()


def _build():
    nc = bacc.Bacc("TRN2", target_bir_lowering=False, debug=False,
                   num_devices=NCORES)

    tensors = {}
    tensors["xT"] = nc.dram_tensor("xT", [NCH * 128, KD, 512], F32,
                                   kind="ExternalInput").ap()
    tensors["xresT"] = nc.dram_tensor("xresT", [TP * 128, KD, 512], BF,
                                      kind="ExternalInput").ap()
    for name, shape, dt in (
        ("wq", [128, KD, 512], BF), ("wk", [128, KD, 512], BF),
        ("wv", [128, KD, 512], BF), ("wo", [128, KHE, D], BF),
        ("w1", [16 * 128, KD, 256], BF), ("w2", [4 * 128, 8, D], BF),
        ("bq", [LHE], F32), ("bk", [LHE], F32), ("bv", [LHE], F32),
        ("b1f", [FF], F32), ("bo2", [D], F32), ("b2", [D], F32),
    ):
        tensors[name] = nc.dram_tensor(name, shape, dt,
                                       kind="ExternalInput").ap()
    tensors["outT"] = nc.dram_tensor("out", [D, LT], F32,
                                     kind="ExternalOutput").ap()

    with tile.TileContext(nc, num_cores=NCORES) as tc:
        _emit(nc, tc, tensors)

    nc.compile()
    return nc


_NC_CACHE = None


def _get_nc():
    global _NC_CACHE
    if _NC_CACHE is None:
        _NC_CACHE = _build()
    return _NC_CACHE


def _shard_inputs(x, Wq, Wk, Wv, Wo, bo, W1, b1, W2, b2, g1, be1, g2, be2):
    """Build the 8 per-core input maps (LN gains folded into weights)."""
    bf = lambda a: np.ascontiguousarray(a).astype(BF16NP)
    f32 = lambda a: np.ascontiguousarray(a, dtype=np.float32)

    x = np.asarray(x, dtype=np.float32)
    Wq = np.asarray(Wq, dtype=np.float32)
    Wk = np.asarray(Wk, dtype=np.float32)
    Wv = np.asarray(Wv, dtype=np.float32)
    Wo = np.asarray(Wo, dtype=np.float32)
    W1 = np.asarray(W1, dtype=np.float32)
    W2 = np.asarray(W2, dtype=np.float32)
    g1 = np.asarray(g1, dtype=np.float32)
    be1 = np.asarray(be1, dtype=np.float32)
    g2 = np.asarray(g2, dtype=np.float32)
    be2 = np.asarray(be2, dtype=np.float32)
    b1 = np.asarray(b1, dtype=np.float32)

    scale = float(HS) ** -0.5
    # fold g1 into QKV weights, be1 into QKV biases; fold the score scale
    # into Wq/bq.  Per-head [H, D, HS] -> concat heads -> [D, H*HS].
    wq_f = (g1[None, :, None] * Wq).transpose(1, 0, 2).reshape(D, D) * scale
    wk_f = (g1[None, :, None] * Wk).transpose(1, 0, 2).reshape(D, D)
    wv_f = (g1[None, :, None] * Wv).transpose(1, 0, 2).reshape(D, D)
    bq_f = np.einsum("d,hde->he", be1, Wq).reshape(D) * scale
    bk_f = np.einsum("d,hde->he", be1, Wk).reshape(D)
    bv_f = np.einsum("d,hde->he", be1, Wv).reshape(D)
    # fold g2/be2 into W1/b1
    w1_f = g2[:, None] * W1
    b1_f = b1 + be2 @ W1

    in_maps = []
    for c in range(NCORES):
        b, half = divmod(c, TP)
        hes = slice(half * LHE, (half + 1) * LHE)
        xt = x[b].T
        xres = np.concatenate(
            [xt[:, ci * 512 + half * 256: ci * 512 + half * 256 + 256]
             for ci in range(NCH)], axis=1)
        # partition-major tiled layouts: loads become 128 contiguous
        # segments instead of 1024 scattered ones (descriptor-gen bound)
        xt_sw = xt.reshape(KD, 128, NCH, 512).transpose(2, 1, 0, 3)
        xres_sw = xres.reshape(KD, 128, TP, 512).transpose(2, 1, 0, 3)
        wq_sw = wq_f[:, hes].reshape(KD, 128, LHE).transpose(1, 0, 2)
        wk_sw = wk_f[:, hes].reshape(KD, 128, LHE).transpose(1, 0, 2)
        wv_sw = wv_f[:, hes].reshape(KD, 128, LHE).transpose(1, 0, 2)
        wo_sw = Wo[hes, :].reshape(KHE, 128, D).transpose(1, 0, 2)
        w1_sw = w1_f.reshape(KD, 128, 16, 256).transpose(2, 1, 0, 3)
        w2_sw = W2.reshape(4, 8, 128, D).transpose(0, 2, 1, 3)
        in_maps.append({
            "xT": f32(xt_sw.reshape(NCH * 128, KD, 512)),
            "xresT": bf(xres_sw.reshape(TP * 128, KD, 512)),
            "wq": bf(wq_sw), "wk": bf(wk_sw), "wv": bf(wv_sw),
            "bq": f32(bq_f[hes]), "bk": f32(bk_f[hes]), "bv": f32(bv_f[hes]),
            "wo": bf(wo_sw),
            "bo2": f32(np.asarray(bo, dtype=np.float32) / TP),
            "w1": bf(w1_sw.reshape(16 * 128, KD, 256)), "b1f": f32(b1_f),
            "w2": bf(w2_sw.reshape(4 * 128, 8, D)),
            "b2": f32(np.asarray(b2, dtype=np.float32)),
        })
    return in_maps


def kernel(x, Wq, Wk, Wv, Wo, bo, W1, b1, W2, b2, g1, be1, g2, be2,
           _trace=False):
    nc = _get_nc()
    in_maps = _shard_inputs(x, Wq, Wk, Wv, Wo, bo, W1, b1, W2, b2,
                            g1, be1, g2, be2)
    res = run_bass_kernel_spmd(nc, in_maps, list(range(NCORES)),
                               trace=_trace)
    out = np.empty((B, T, D), dtype=np.float32)
    for b in range(B):
        for half in range(TP):
            o = res.results[TP * b + half]["out"]  # [D, LT]
            for ci in range(NCH):
                t0 = ci * 512 + half * 256
                out[b, t0:t0 + 256, :] = o[:, ci * 256:(ci + 1) * 256].T
    if _trace:
        kernel.last_exec_time_ns = res.exec_time_ns
        kernel.last_results = res
    return out
